# revision 32
# baseline (speedup 1.0000x reference)
"""Trainium2 Bass kernel for a hypernetwork-generated per-case MLP.

Math (fp32 reference):
  h = silu(o @ Wc + bc)                        [C=64, H=256]
  w = einsum('ch,lhd->lcd', h, Ww) + bw        [L=4, C, 65536]
  b = einsum('ch,lhd->lcd', h, Wb) + bb        [L=4, C, 256]
  per-case 4-layer MLP over shared x [2048, 256] with silu + skip:
    a0 = silu(x @ W0 + b0); a1 = silu(a0 @ W1 + b1)
    a2 = silu(a1 @ W2 + b2); out = (a2 + a0) @ W3 + b3
  returns [C*N, 256]

Distribution over 8 NeuronCores:
  - weight-gen tensor-sharded over the d axis of Ww (each core owns a
    contiguous 8192-wide shard, computes w[:, all 64 cases, shard] with
    col-tiled M=64 matmul pairs);
  - per-layer AllToAll redistributes w so core k holds full-d weights for
    its 8 cases;
  - domain net data-parallel over cases (8 per core), run as 2 groups of
    4 cases x layer-outer so each AllToAll hides behind the previous
    layer's compute; activations feature-major [feat, n] in SBUF.
  - engine split: PE matmuls; ACT does all silu (the throughput floor);
    final-layer drains alternate ACT/DVE; skip adds + weight bias adds
    on DVE; weight-gen psum drains on DVE.
  - all weight-gen issues before the domain loop (the per-layer AllToAlls
    then serialize behind a dummy collective that soaks the CC firmware's
    ~70us cold start); lhsT stationary reused across 4 matmuls.
"""

import numpy as np

import concourse.bass as bass
import concourse.mybir as mybir
import concourse.tile as tile
from concourse import bacc
from concourse.bass import ts, ds
from concourse.bass_utils import run_bass_kernel_spmd

F32 = mybir.dt.float32
F16 = mybir.dt.float16
AF = mybir.ActivationFunctionType

P = 128
NCORES = 8
C = 64          # total cases
CC = C // NCORES  # cases per core
CIN = 64        # caseNN input dim
H = 256         # caseNN hidden
HB = H // P     # h k-blocks (2)
DIN = 256       # domain feature dim (in = out = 256 for every layer)
IB = DIN // P   # 2
NL = 4          # layers
N = 2048        # samples
D = DIN * DIN   # 65536 flattened per-layer weight
DSH = D // NCORES  # 8192 per-core d shard
QD = DSH // 4   # 2048-wide quarters of the shard
GRP = 4         # cases per domain group
NGRP = CC // GRP
NSL = 2         # n-slots of 1024 per (case, layer, ob)
_nc_cache = {}


def _build():
    nc = bacc.Bacc("TRN2", target_bir_lowering=False, debug=False, num_devices=NCORES)

    # ---- per-core external I/O ----
    xt = nc.dram_tensor("xt", [P, IB, N], F16, kind="ExternalInput").ap()
    ot = nc.dram_tensor("ot", [P, C], F16, kind="ExternalInput").ap()
    oto = nc.dram_tensor("oto", [P, CC], F16, kind="ExternalInput").ap()
    wc = nc.dram_tensor("wc", [P, H], F16, kind="ExternalInput").ap()
    bc2 = nc.dram_tensor("bc2", [P, HB], F32, kind="ExternalInput").ap()
    wws = nc.dram_tensor("wws", [NL, H, DSH], F16, kind="ExternalInput").ap()
    wbT = nc.dram_tensor("wbT", [P, HB, NL, DIN], F16, kind="ExternalInput").ap()
    bbT = nc.dram_tensor("bbT", [P, IB, NL], F32, kind="ExternalInput").ap()
    bwT = nc.dram_tensor("bwT", [P, NL, IB, DIN], F16, kind="ExternalInput").ap()
    yt = nc.dram_tensor("yt", [CC, IB, P, N], F16, kind="ExternalOutput").ap()

    with tile.TileContext(nc) as tc:
        with (
            tc.tile_pool(name="const", bufs=1) as const,
            tc.tile_pool(name="dram", bufs=1, space="DRAM") as dram,
            tc.tile_pool(name="ww", bufs=6) as ww,
            tc.tile_pool(name="wstg", bufs=4) as wstg,
            tc.tile_pool(name="wt", bufs=16) as wtp,
            tc.tile_pool(name="act", bufs=1) as act,
            tc.tile_pool(name="outs", bufs=2) as outs,
        ):
            # caseNN + weight-gen psum (2 banks); closed before the domain
            # pool opens so the domain gets all 8 banks
            ps_w_ctx = tc.tile_pool(name="ps_w", bufs=2, space="PSUM")
            ps_w = ps_w_ctx.__enter__()
            # ---- dummy collective: absorb the one-time CC firmware cold
            # start (~40us) concurrently with weight-gen ----
            cc_warm_in = dram.tile([NCORES, 64], F16, name="cc_warm_in")
            cc_warm_out = dram.tile([NCORES, 64], F16, name="cc_warm_out")
            nc.gpsimd.collective_compute(
                "AllToAll",
                mybir.AluOpType.bypass,
                replica_groups=[list(range(NCORES))],
                ins=[cc_warm_in.opt()],
                outs=[cc_warm_out.opt()],
            )

            # ---- tiny consts first: keep the wgen(0) critical path clear ----
            wc_sb = const.tile([P, H], F16)
            nc.sync.dma_start(wc_sb[:], wc)
            bc_sb = const.tile([P, HB], F32)
            nc.sync.dma_start(bc_sb[:], bc2)
            ot_sb = const.tile([P, C], F16)
            nc.sync.dma_start(ot_sb[:], ot)
            oto_sb = const.tile([P, CC], F16)
            nc.sync.dma_start(oto_sb[:], oto)

            # ---- PE warm-up: drive HAM to K=8/8 before weight-gen ----
            warm = ps_w.tile([P, 512], F32, tag="psw", name="warm")
            for i in range(24):
                nc.tensor.matmul(warm[:, :256], lhsT=wc_sb[:, 0:P],
                                 rhs=wc_sb, start=True, stop=True)

            # ---- caseNN hidden: hT[h, c] = silu(Wc.T @ o.T + bc) ----
            hT_sb = const.tile([P, HB, C], F16)
            for kb in range(HB):
                ps = ps_w.tile([P, 512], F32, tag="psw", name="psh")[:, :C]
                nc.tensor.matmul(ps, lhsT=wc_sb[:, ts(kb, P)], rhs=ot_sb,
                                 start=True, stop=True)
                nc.scalar.activation(hT_sb[:, kb, :], ps, AF.Silu,
                                     bias=bc_sb[:, kb : kb + 1])

            # ---- DRAM staging for collectives: {l0+l1}, {l2}, {l3} so the
            # first AllToAll delivers two layers and the chain stays ahead
            # of the domain windows ----
            w_sh01 = dram.tile([C, 2 * DSH], F16, name="w_sh01")
            w_fl01 = dram.tile([C, 2 * DSH], F16, name="w_fl01")
            w_sh2 = dram.tile([C, DSH], F16, name="w_sh2")
            w_fl2 = dram.tile([C, DSH], F16, name="w_fl2")
            w_sh3 = dram.tile([C, DSH], F16, name="w_sh3")
            w_fl3 = dram.tile([C, DSH], F16, name="w_fl3")
            # rows: j*CC + c_loc (j = source core = d-shard index);
            # d global = i*256 + o, shard j covers i in [32j, 32j+32)
            v01 = w_fl01.rearrange("(j c) (l il o) -> l j c il o", c=CC, l=2, o=DIN)
            v2 = w_fl2.rearrange("(j c) (il o) -> j c il o", c=CC, o=DIN)
            v3 = w_fl3.rearrange("(j c) (il o) -> j c il o", c=CC, o=DIN)

            def wf_view(l, j, c):
                if l < 2:
                    return v01[l, j, c]
                return (v2 if l == 2 else v3)[j, c]

            def shard_dst(l, off, size):
                if l < 2:
                    return w_sh01[:, ds(l * DSH + off, size)]
                return (w_sh2 if l == 2 else w_sh3)[:, ds(off, size)]

            def wgen(l):
                """weight-gen layer l: w[c, d-shard] for all 64 cases,
                col-tiled M=64 matmul pairs into [128, 512] psum tiles."""
                wws_l = wws[l].rearrange("(kb p) d -> p kb d", p=P)
                for q in range(4):
                    wwt = ww.tile([P, HB, QD], F16, tag="wwt", name=f"wwt{l}{q}")
                    nc.sync.dma_start(wwt[:], wws_l[:, :, ts(q, QD)])
                    for pr in range(2):  # chunk-pairs (2 x 512) per quarter
                        ps = ps_w.tile([P, 512], F32, tag="psw", name=f"psw{l}{q}{pr}")
                        for kb in range(HB):
                            nc.tensor.matmul(
                                ps[0:64, :], lhsT=hT_sb[:, kb, :],
                                rhs=wwt[:, kb, ds(pr * 1024, 512)],
                                start=(kb == 0), stop=(kb == HB - 1),
                            )
                            nc.tensor.matmul(
                                ps[64:128, :], lhsT=hT_sb[:, kb, :],
                                rhs=wwt[:, kb, ds(pr * 1024 + 512, 512)],
                                start=(kb == 0), stop=(kb == HB - 1),
                            )
                        stg = wstg.tile([P, 512], F16, tag="wstg", name=f"stg{l}{q}{pr}")
                        nc.vector.tensor_copy(stg[:], ps)
                        base = q * QD + pr * 1024
                        nc.sync.dma_start(shard_dst(l, base, 512), stg[0:64, :])
                        nc.sync.dma_start(shard_dst(l, base + 512, 512), stg[64:128, :])
                if l != 0:
                    ins_t, outs_t = {1: (w_sh01, w_fl01), 2: (w_sh2, w_fl2),
                                     3: (w_sh3, w_fl3)}[l]
                    nc.gpsimd.collective_compute(
                        "AllToAll",
                        mybir.AluOpType.bypass,
                        replica_groups=[list(range(NCORES))],
                        ins=[ins_t.opt()],
                        outs=[outs_t.opt()],
                    )

            # domain state per case
            a_cur = [None] * CC   # input tile for next layer
            a_skip = [None] * CC  # a0 (skip accumulator, f16)
            wt_tiles = {}

            def prep(g, l):
                """DMA-gather + bias-add the domain weight tiles for group g
                layer l (4 cases x 2 ib-tiles of [128, 256] f16)."""
                # group 0 bias-adds on DVE (free early); group 1 on GpSimd
                # (collectives done by then) so l3 DVE drains never block them
                eng = nc.vector if g == 0 else nc.gpsimd
                for c in range(g * GRP, (g + 1) * GRP):
                    for ib in range(IB):
                        wt_t = wtp.tile([P, DIN], F16, tag="wt", name=f"wt{g}{l}{c}{ib}")
                        for jr in range(4):
                            j = 4 * ib + jr
                            nc.sync.dma_start(wt_t[ds(32 * jr, 32), :],
                                              wf_view(l, j, c))
                        eng.tensor_add(wt_t[:], wt_t[:], bwT_sb[:, l, ib, :])
                        wt_tiles[(c, ib)] = wt_t

            def compute(g, l):
                """domain layer l for the 4 cases of group g."""
                last_c = (g + 1) * GRP - 1
                for c in range(g * GRP, (g + 1) * GRP):
                    a_prev = xt_sb if l == 0 else a_cur[c]
                    if l == 0:
                        a_new = act.tile([P, IB, N], F16, tag="a0", bufs=4,
                                         name=f"a0_{c}")
                    elif l < NL - 1:
                        a_new = act.tile([P, IB, N], F16, tag="ax", bufs=8,
                                         name=f"a_{c}_{l}")
                    else:
                        a_new = outs.tile([P, IB, N], F16, tag="out",
                                          name=f"o_{c}")
                    for ob in range(IB):
                        # one 4-bank psum tile per ob: each lhsT (stationary
                        # weight) covers 4 consecutive matmuls, and the whole
                        # 2048-wide tile drains in ONE ACT/DVE instruction
                        ps = ps_y.tile([P, 2048], F32, tag="psy",
                                       name=f"psy{c}{l}{ob}")
                        for ib in range(IB):
                            for nch in range(4):
                                nc.tensor.matmul(
                                    ps[:, ts(nch, 512)],
                                    lhsT=wt_tiles[(c, ib)][:, ts(ob, P)],
                                    rhs=a_prev[:, ib, ds(nch * 512, 512)],
                                    start=(ib == 0), stop=(ib == IB - 1),
                                )
                        bias = bO_sb[:, ob, l, c : c + 1]
                        # the window's last case drains in 1024-halves so the
                        # next window's matmuls (subtile WAR on this psum
                        # slot) can restart ~1us sooner
                        spans = ((0, 1024), (1024, 1024)) if c == last_c \
                            else ((0, 2048),)
                        for (off, sz) in spans:
                            dst = a_new[:, ob, ds(off, sz)]
                            pss = ps[:, ds(off, sz)]
                            if l < NL - 1:
                                nc.scalar.activation(dst, pss, AF.Silu, bias=bias)
                            elif ob == 0:
                                nc.scalar.activation(dst, pss, AF.Identity, bias=bias)
                            else:
                                nc.vector.tensor_scalar_add(dst, pss, bias)
                        if l == NL - 1:
                            nc.sync.dma_start(yt[c, ob], a_new[:, ob, :])
                    if l == 0:
                        a_skip[c] = a_new
                    if l == 2:  # skip: a0 += a2, feeds final layer
                        nc.vector.tensor_add(a_skip[c][:], a_skip[c][:], a_new[:])
                        a_new = a_skip[c]
                    a_cur[c] = a_new

            # ---- braided issue: wgen layer l+1 interleaves domain layer l ----
            wgen(0)
            wgen(1)
            wgen(2)
            wgen(3)

            # big consts stream after all Ww quarters have queue priority
            xt_sb = const.tile([P, IB, N], F16)
            nc.sync.dma_start(xt_sb[:], xt)
            wbT_sb = const.tile([P, HB, NL, DIN], F16)
            nc.sync.dma_start(wbT_sb[:], wbT)
            bbT_sb = const.tile([P, IB, NL], F32)
            nc.sync.dma_start(bbT_sb[:], bbT)
            bwT_sb = const.tile([P, NL, IB, DIN], F16)
            nc.sync.dma_start(bwT_sb[:], bwT)

            # ---- caseNN for own cases: hTo + per-layer bias bO[o, ob, l, c] ----
            hTo_sb = const.tile([P, HB, CC], F16)
            for kb in range(HB):
                ps2 = ps_w.tile([P, 512], F32, tag="psw", name="psh2")[:, :CC]
                nc.tensor.matmul(ps2, lhsT=wc_sb[:, ts(kb, P)], rhs=oto_sb,
                                 start=True, stop=True)
                nc.scalar.activation(hTo_sb[:, kb, :], ps2, AF.Silu,
                                     bias=bc_sb[:, kb : kb + 1])
            bO_sb = const.tile([P, IB, NL, CC], F32)
            for l in range(NL):
                for ob in range(IB):
                    ps = ps_w.tile([P, 512], F32, tag="psw", name="psb")[:, :CC]
                    for kb in range(HB):
                        nc.tensor.matmul(
                            ps, lhsT=wbT_sb[:, kb, l, ts(ob, P)],
                            rhs=hTo_sb[:, kb, :],
                            start=(kb == 0), stop=(kb == HB - 1),
                        )
                    nc.scalar.activation(
                        bO_sb[:, ob, l, :], ps, AF.Identity,
                        bias=bbT_sb[:, ob, l : l + 1]
                    )

            ps_w_ctx.__exit__(None, None, None)
            ps_y_ctx = tc.tile_pool(name="ps_y", bufs=2, space="PSUM")
            ps_y = ps_y_ctx.__enter__()
            for g in range(NGRP):
                for l in range(NL):
                    prep(g, l)
                    compute(g, l)
            ps_y_ctx.__exit__(None, None, None)

    nc.compile()
    return nc


def _prep_inputs(x, o, Wc, bc, Ww, bw, Wb, bb):
    x = np.asarray(x, np.float32)
    o = np.asarray(o, np.float32)
    Wc = np.asarray(Wc, np.float32)
    bc = np.asarray(bc, np.float32)
    Ww = np.asarray(Ww, np.float32)
    bw = np.asarray(bw, np.float32)
    Wb = np.asarray(Wb, np.float32)
    bb = np.asarray(bb, np.float32)

    xt = np.ascontiguousarray(x.T.reshape(IB, P, N).transpose(1, 0, 2)).astype(np.float16)
    otf = np.zeros((P, C), np.float16)
    otf[:CIN, :] = o.T
    wcp = np.zeros((P, H), np.float16)
    wcp[:CIN, :] = Wc
    bc2 = np.ascontiguousarray(bc.reshape(HB, P).T)
    wbT = np.ascontiguousarray(Wb.reshape(NL, HB, P, DIN).transpose(2, 1, 0, 3)).astype(np.float16)
    bbT = np.ascontiguousarray(bb.reshape(NL, IB, P).transpose(2, 1, 0))
    bwT = np.ascontiguousarray(bw.reshape(NL, IB, P, DIN).transpose(2, 0, 1, 3)).astype(np.float16)

    in_maps = []
    for k in range(NCORES):
        in_maps.append(
            {
                "xt": xt,
                "ot": otf,
                "oto": np.ascontiguousarray(otf[:, k * CC : (k + 1) * CC]),
                "wc": wcp,
                "bc2": bc2,
                "wws": np.ascontiguousarray(Ww[:, :, k * DSH : (k + 1) * DSH]).astype(np.float16),
                "wbT": wbT,
                "bbT": bbT,
                "bwT": bwT,
            }
        )
    return in_maps


def _run(inputs, trace=False):
    if "nc" not in _nc_cache:
        _nc_cache["nc"] = _build()
    nc = _nc_cache["nc"]
    in_maps = _prep_inputs(**inputs)
    res = run_bass_kernel_spmd(
        nc, in_maps, core_ids=list(range(NCORES)), trace=trace
    )
    # yt per core: [CC, IB, P, N] f16 -> [CC, N, IB*P] case-major
    parts = []
    for k in range(NCORES):
        ytk = res.results[k]["yt"].astype(np.float32)
        parts.append(ytk.transpose(0, 3, 1, 2).reshape(CC, N, DIN))
    out = np.concatenate(parts, axis=0).reshape(C * N, DIN)
    return out, res


def kernel(**inputs):
    out, _ = _run(inputs, trace=False)
    return out


# revision 33
# speedup vs baseline: 1.0090x; 1.0090x over previous
"""Trainium2 Bass kernel for a hypernetwork-generated per-case MLP.

Math (fp32 reference):
  h = silu(o @ Wc + bc)                        [C=64, H=256]
  w = einsum('ch,lhd->lcd', h, Ww) + bw        [L=4, C, 65536]
  b = einsum('ch,lhd->lcd', h, Wb) + bb        [L=4, C, 256]
  per-case 4-layer MLP over shared x [2048, 256] with silu + skip:
    a0 = silu(x @ W0 + b0); a1 = silu(a0 @ W1 + b1)
    a2 = silu(a1 @ W2 + b2); out = (a2 + a0) @ W3 + b3
  returns [C*N, 256]

Distribution over 8 NeuronCores:
  - weight-gen tensor-sharded over the d axis of Ww (each core owns a
    contiguous 8192-wide shard, computes w[:, all 64 cases, shard] with
    col-tiled M=64 matmul pairs);
  - per-layer AllToAll redistributes w so core k holds full-d weights for
    its 8 cases;
  - domain net data-parallel over cases (8 per core), run as 2 groups of
    4 cases x layer-outer so each AllToAll hides behind the previous
    layer's compute; activations feature-major [feat, n] in SBUF.
  - engine split: PE matmuls; ACT does all silu (the throughput floor);
    final-layer drains alternate ACT/DVE; skip adds + weight bias adds
    on DVE; weight-gen psum drains on DVE.
  - all weight-gen issues before the domain loop (the per-layer AllToAlls
    then serialize behind a dummy collective that soaks the CC firmware's
    ~70us cold start); lhsT stationary reused across 4 matmuls.
"""

import numpy as np

import concourse.bass as bass
import concourse.mybir as mybir
import concourse.tile as tile
from concourse import bacc
from concourse.bass import ts, ds
from concourse.bass_utils import run_bass_kernel_spmd

F32 = mybir.dt.float32
F16 = mybir.dt.float16
AF = mybir.ActivationFunctionType

P = 128
NCORES = 8
C = 64          # total cases
CC = C // NCORES  # cases per core
CIN = 64        # caseNN input dim
H = 256         # caseNN hidden
HB = H // P     # h k-blocks (2)
DIN = 256       # domain feature dim (in = out = 256 for every layer)
IB = DIN // P   # 2
NL = 4          # layers
N = 2048        # samples
D = DIN * DIN   # 65536 flattened per-layer weight
DSH = D // NCORES  # 8192 per-core d shard
QD = DSH // 4   # 2048-wide quarters of the shard
GRP = 4         # cases per domain group
NGRP = CC // GRP
NSL = 2         # n-slots of 1024 per (case, layer, ob)
_nc_cache = {}


def _build():
    nc = bacc.Bacc("TRN2", target_bir_lowering=False, debug=False, num_devices=NCORES)

    # ---- per-core external I/O ----
    xt = nc.dram_tensor("xt", [P, IB, N], F16, kind="ExternalInput").ap()
    ot = nc.dram_tensor("ot", [P, C], F16, kind="ExternalInput").ap()
    oto = nc.dram_tensor("oto", [P, CC], F16, kind="ExternalInput").ap()
    wc = nc.dram_tensor("wc", [P, H], F16, kind="ExternalInput").ap()
    bc2 = nc.dram_tensor("bc2", [P, HB], F32, kind="ExternalInput").ap()
    wws = nc.dram_tensor("wws", [NL, H, DSH], F16, kind="ExternalInput").ap()
    wbT = nc.dram_tensor("wbT", [P, HB, NL, DIN], F16, kind="ExternalInput").ap()
    bbT = nc.dram_tensor("bbT", [P, IB, NL], F32, kind="ExternalInput").ap()
    bwT = nc.dram_tensor("bwT", [P, NL, IB, DIN], F16, kind="ExternalInput").ap()
    yt = nc.dram_tensor("yt", [CC, IB, P, N], F16, kind="ExternalOutput").ap()

    with tile.TileContext(nc) as tc:
        with (
            tc.tile_pool(name="const", bufs=1) as const,
            tc.tile_pool(name="dram", bufs=1, space="DRAM") as dram,
            tc.tile_pool(name="ww", bufs=6) as ww,
            tc.tile_pool(name="wstg", bufs=4) as wstg,
            tc.tile_pool(name="wt", bufs=16) as wtp,
            tc.tile_pool(name="act", bufs=1) as act,
            tc.tile_pool(name="outs", bufs=2) as outs,
        ):
            # caseNN + weight-gen psum (2 banks); closed before the domain
            # pool opens so the domain gets all 8 banks
            ps_w_ctx = tc.tile_pool(name="ps_w", bufs=2, space="PSUM")
            ps_w = ps_w_ctx.__enter__()
            # ---- dummy collective: absorb the one-time CC firmware cold
            # start (~40us) concurrently with weight-gen ----
            cc_warm_in = dram.tile([NCORES, 64], F16, name="cc_warm_in")
            cc_warm_out = dram.tile([NCORES, 64], F16, name="cc_warm_out")
            nc.gpsimd.collective_compute(
                "AllToAll",
                mybir.AluOpType.bypass,
                replica_groups=[list(range(NCORES))],
                ins=[cc_warm_in.opt()],
                outs=[cc_warm_out.opt()],
            )

            # ---- tiny consts first: keep the wgen(0) critical path clear ----
            wc_sb = const.tile([P, H], F16)
            nc.sync.dma_start(wc_sb[:], wc)
            bc_sb = const.tile([P, HB], F32)
            nc.sync.dma_start(bc_sb[:], bc2)
            ot_sb = const.tile([P, C], F16)
            nc.sync.dma_start(ot_sb[:], ot)
            oto_sb = const.tile([P, CC], F16)
            nc.sync.dma_start(oto_sb[:], oto)

            # ---- PE warm-up: drive HAM to K=8/8 before weight-gen ----
            warm = ps_w.tile([P, 512], F32, tag="psw", name="warm")
            for i in range(24):
                nc.tensor.matmul(warm[:, :256], lhsT=wc_sb[:, 0:P],
                                 rhs=wc_sb, start=True, stop=True)

            # ---- caseNN hidden: hT[h, c] = silu(Wc.T @ o.T + bc) ----
            hT_sb = const.tile([P, HB, C], F16)
            for kb in range(HB):
                ps = ps_w.tile([P, 512], F32, tag="psw", name="psh")[:, :C]
                nc.tensor.matmul(ps, lhsT=wc_sb[:, ts(kb, P)], rhs=ot_sb,
                                 start=True, stop=True)
                nc.scalar.activation(hT_sb[:, kb, :], ps, AF.Silu,
                                     bias=bc_sb[:, kb : kb + 1])

            # ---- DRAM staging for collectives: {l0+l1}, {l2}, {l3} so the
            # first AllToAll delivers two layers and the chain stays ahead
            # of the domain windows ----
            w_sh01 = dram.tile([C, 2 * DSH], F16, name="w_sh01")
            w_fl01 = dram.tile([C, 2 * DSH], F16, name="w_fl01")
            w_sh2 = dram.tile([C, DSH], F16, name="w_sh2")
            w_fl2 = dram.tile([C, DSH], F16, name="w_fl2")
            w_sh3 = dram.tile([C, DSH], F16, name="w_sh3")
            w_fl3 = dram.tile([C, DSH], F16, name="w_fl3")
            # rows: j*CC + c_loc (j = source core = d-shard index);
            # d global = i*256 + o, shard j covers i in [32j, 32j+32)
            v01 = w_fl01.rearrange("(j c) (l il o) -> l j c il o", c=CC, l=2, o=DIN)
            v2 = w_fl2.rearrange("(j c) (il o) -> j c il o", c=CC, o=DIN)
            v3 = w_fl3.rearrange("(j c) (il o) -> j c il o", c=CC, o=DIN)

            def wf_view(l, j, c):
                if l < 2:
                    return v01[l, j, c]
                return (v2 if l == 2 else v3)[j, c]

            def shard_dst(l, off, size):
                if l < 2:
                    return w_sh01[:, ds(l * DSH + off, size)]
                return (w_sh2 if l == 2 else w_sh3)[:, ds(off, size)]

            def wgen(l):
                """weight-gen layer l: w[c, d-shard] for all 64 cases,
                col-tiled M=64 matmul pairs into [128, 512] psum tiles."""
                wws_l = wws[l].rearrange("(kb p) d -> p kb d", p=P)
                for q in range(4):
                    wwt = ww.tile([P, HB, QD], F16, tag="wwt", name=f"wwt{l}{q}")
                    nc.sync.dma_start(wwt[:], wws_l[:, :, ts(q, QD)])
                    for pr in range(2):  # chunk-pairs (2 x 512) per quarter
                        ps = ps_w.tile([P, 512], F32, tag="psw", name=f"psw{l}{q}{pr}")
                        for kb in range(HB):
                            nc.tensor.matmul(
                                ps[0:64, :], lhsT=hT_sb[:, kb, :],
                                rhs=wwt[:, kb, ds(pr * 1024, 512)],
                                start=(kb == 0), stop=(kb == HB - 1),
                            )
                            nc.tensor.matmul(
                                ps[64:128, :], lhsT=hT_sb[:, kb, :],
                                rhs=wwt[:, kb, ds(pr * 1024 + 512, 512)],
                                start=(kb == 0), stop=(kb == HB - 1),
                            )
                        stg = wstg.tile([P, 512], F16, tag="wstg", name=f"stg{l}{q}{pr}")
                        nc.vector.tensor_copy(stg[:], ps)
                        base = q * QD + pr * 1024
                        nc.sync.dma_start(shard_dst(l, base, 512), stg[0:64, :])
                        nc.sync.dma_start(shard_dst(l, base + 512, 512), stg[64:128, :])
                if l != 0:
                    ins_t, outs_t = {1: (w_sh01, w_fl01), 2: (w_sh2, w_fl2),
                                     3: (w_sh3, w_fl3)}[l]
                    nc.gpsimd.collective_compute(
                        "AllToAll",
                        mybir.AluOpType.bypass,
                        replica_groups=[list(range(NCORES))],
                        ins=[ins_t.opt()],
                        outs=[outs_t.opt()],
                    )

            # domain state per case
            a_cur = [None] * CC   # input tile for next layer
            a_skip = [None] * CC  # a0 (skip accumulator, f16)
            wt_tiles = {}

            def prep(g, l):
                """DMA-gather + bias-add the domain weight tiles for group g
                layer l (4 cases x 2 ib-tiles of [128, 256] f16)."""
                # group 0 bias-adds on DVE (free early); group 1 on GpSimd
                # (collectives done by then) so l3 DVE drains never block them
                eng = nc.vector if g == 0 else nc.gpsimd
                for c in range(g * GRP, (g + 1) * GRP):
                    for ib in range(IB):
                        wt_t = wtp.tile([P, DIN], F16, tag="wt", name=f"wt{g}{l}{c}{ib}")
                        for jr in range(4):
                            j = 4 * ib + jr
                            nc.sync.dma_start(wt_t[ds(32 * jr, 32), :],
                                              wf_view(l, j, c))
                        eng.tensor_add(wt_t[:], wt_t[:], bwT_sb[:, l, ib, :])
                        wt_tiles[(c, ib)] = wt_t

            def compute(g, l):
                """domain layer l for the 4 cases of group g."""
                for c in range(g * GRP, (g + 1) * GRP):
                    a_prev = xt_sb if l == 0 else a_cur[c]
                    if l == 0:
                        a_new = act.tile([P, IB, N], F16, tag="a0", bufs=4,
                                         name=f"a0_{c}")
                    elif l < NL - 1:
                        a_new = act.tile([P, IB, N], F16, tag="ax", bufs=8,
                                         name=f"a_{c}_{l}")
                    else:
                        a_new = outs.tile([P, IB, N], F16, tag="out",
                                          name=f"o_{c}")
                    for ob in range(IB):
                        # one 4-bank psum tile per ob: each lhsT (stationary
                        # weight) covers 4 consecutive matmuls, and the whole
                        # 2048-wide tile drains in ONE ACT/DVE instruction
                        ps = ps_y.tile([P, 2048], F32, tag="psy",
                                       name=f"psy{c}{l}{ob}")
                        for ib in range(IB):
                            for nch in range(4):
                                nc.tensor.matmul(
                                    ps[:, ts(nch, 512)],
                                    lhsT=wt_tiles[(c, ib)][:, ts(ob, P)],
                                    rhs=a_prev[:, ib, ds(nch * 512, 512)],
                                    start=(ib == 0), stop=(ib == IB - 1),
                                )
                        dst = a_new[:, ob, :]
                        bias = bO_sb[:, ob, l, c : c + 1]
                        if l < NL - 1:
                            nc.scalar.activation(dst, ps, AF.Silu, bias=bias)
                        else:
                            nc.scalar.activation(dst, ps, AF.Identity, bias=bias)
                        if l == NL - 1:
                            nc.sync.dma_start(yt[c, ob], a_new[:, ob, :])
                    if l == 0:
                        a_skip[c] = a_new
                    if l == 2:  # skip: a0 += a2, feeds final layer
                        nc.vector.tensor_add(a_skip[c][:], a_skip[c][:], a_new[:])
                        a_new = a_skip[c]
                    a_cur[c] = a_new

            # ---- braided issue: wgen layer l+1 interleaves domain layer l ----
            wgen(0)
            wgen(1)
            wgen(2)
            wgen(3)

            # big consts stream after all Ww quarters have queue priority
            xt_sb = const.tile([P, IB, N], F16)
            nc.sync.dma_start(xt_sb[:], xt)
            wbT_sb = const.tile([P, HB, NL, DIN], F16)
            nc.sync.dma_start(wbT_sb[:], wbT)
            bbT_sb = const.tile([P, IB, NL], F32)
            nc.sync.dma_start(bbT_sb[:], bbT)
            bwT_sb = const.tile([P, NL, IB, DIN], F16)
            nc.sync.dma_start(bwT_sb[:], bwT)

            # ---- caseNN for own cases: hTo + per-layer bias bO[o, ob, l, c] ----
            hTo_sb = const.tile([P, HB, CC], F16)
            for kb in range(HB):
                ps2 = ps_w.tile([P, 512], F32, tag="psw", name="psh2")[:, :CC]
                nc.tensor.matmul(ps2, lhsT=wc_sb[:, ts(kb, P)], rhs=oto_sb,
                                 start=True, stop=True)
                nc.scalar.activation(hTo_sb[:, kb, :], ps2, AF.Silu,
                                     bias=bc_sb[:, kb : kb + 1])
            bO_sb = const.tile([P, IB, NL, CC], F32)
            for l in range(NL):
                for ob in range(IB):
                    ps = ps_w.tile([P, 512], F32, tag="psw", name="psb")[:, :CC]
                    for kb in range(HB):
                        nc.tensor.matmul(
                            ps, lhsT=wbT_sb[:, kb, l, ts(ob, P)],
                            rhs=hTo_sb[:, kb, :],
                            start=(kb == 0), stop=(kb == HB - 1),
                        )
                    nc.scalar.activation(
                        bO_sb[:, ob, l, :], ps, AF.Identity,
                        bias=bbT_sb[:, ob, l : l + 1]
                    )

            ps_w_ctx.__exit__(None, None, None)
            ps_y_ctx = tc.tile_pool(name="ps_y", bufs=2, space="PSUM")
            ps_y = ps_y_ctx.__enter__()
            for g in range(NGRP):
                for l in range(NL):
                    prep(g, l)
                    compute(g, l)
            ps_y_ctx.__exit__(None, None, None)

    nc.compile()
    return nc


def _prep_inputs(x, o, Wc, bc, Ww, bw, Wb, bb):
    x = np.asarray(x, np.float32)
    o = np.asarray(o, np.float32)
    Wc = np.asarray(Wc, np.float32)
    bc = np.asarray(bc, np.float32)
    Ww = np.asarray(Ww, np.float32)
    bw = np.asarray(bw, np.float32)
    Wb = np.asarray(Wb, np.float32)
    bb = np.asarray(bb, np.float32)

    xt = np.ascontiguousarray(x.T.reshape(IB, P, N).transpose(1, 0, 2)).astype(np.float16)
    otf = np.zeros((P, C), np.float16)
    otf[:CIN, :] = o.T
    wcp = np.zeros((P, H), np.float16)
    wcp[:CIN, :] = Wc
    bc2 = np.ascontiguousarray(bc.reshape(HB, P).T)
    wbT = np.ascontiguousarray(Wb.reshape(NL, HB, P, DIN).transpose(2, 1, 0, 3)).astype(np.float16)
    bbT = np.ascontiguousarray(bb.reshape(NL, IB, P).transpose(2, 1, 0))
    bwT = np.ascontiguousarray(bw.reshape(NL, IB, P, DIN).transpose(2, 0, 1, 3)).astype(np.float16)

    in_maps = []
    for k in range(NCORES):
        in_maps.append(
            {
                "xt": xt,
                "ot": otf,
                "oto": np.ascontiguousarray(otf[:, k * CC : (k + 1) * CC]),
                "wc": wcp,
                "bc2": bc2,
                "wws": np.ascontiguousarray(Ww[:, :, k * DSH : (k + 1) * DSH]).astype(np.float16),
                "wbT": wbT,
                "bbT": bbT,
                "bwT": bwT,
            }
        )
    return in_maps


def _run(inputs, trace=False):
    if "nc" not in _nc_cache:
        _nc_cache["nc"] = _build()
    nc = _nc_cache["nc"]
    in_maps = _prep_inputs(**inputs)
    res = run_bass_kernel_spmd(
        nc, in_maps, core_ids=list(range(NCORES)), trace=trace
    )
    # yt per core: [CC, IB, P, N] f16 -> [CC, N, IB*P] case-major
    parts = []
    for k in range(NCORES):
        ytk = res.results[k]["yt"].astype(np.float32)
        parts.append(ytk.transpose(0, 3, 1, 2).reshape(CC, N, DIN))
    out = np.concatenate(parts, axis=0).reshape(C * N, DIN)
    return out, res


def kernel(**inputs):
    out, _ = _run(inputs, trace=False)
    return out


# revision 34
# speedup vs baseline: 1.0257x; 1.0166x over previous
"""Trainium2 Bass kernel for a hypernetwork-generated per-case MLP.

Math (fp32 reference):
  h = silu(o @ Wc + bc)                        [C=64, H=256]
  w = einsum('ch,lhd->lcd', h, Ww) + bw        [L=4, C, 65536]
  b = einsum('ch,lhd->lcd', h, Wb) + bb        [L=4, C, 256]
  per-case 4-layer MLP over shared x [2048, 256] with silu + skip:
    a0 = silu(x @ W0 + b0); a1 = silu(a0 @ W1 + b1)
    a2 = silu(a1 @ W2 + b2); out = (a2 + a0) @ W3 + b3
  returns [C*N, 256]

Distribution over 8 NeuronCores:
  - weight-gen tensor-sharded over the d axis of Ww (each core owns a
    contiguous 8192-wide shard, computes w[:, all 64 cases, shard] with
    col-tiled M=64 matmul pairs);
  - per-layer AllToAll redistributes w so core k holds full-d weights for
    its 8 cases;
  - domain net data-parallel over cases (8 per core), run as 2 groups of
    4 cases x layer-outer so each AllToAll hides behind the previous
    layer's compute; activations feature-major [feat, n] in SBUF.
  - engine split: PE matmuls; ACT does all silu (the throughput floor);
    final-layer drains alternate ACT/DVE; skip adds + weight bias adds
    on DVE; weight-gen psum drains on DVE.
  - all weight-gen issues before the domain loop (the per-layer AllToAlls
    then serialize behind a dummy collective that soaks the CC firmware's
    ~70us cold start); lhsT stationary reused across 4 matmuls.
"""

import numpy as np

import concourse.bass as bass
import concourse.mybir as mybir
import concourse.tile as tile
from concourse import bacc
from concourse.bass import ts, ds
from concourse.bass_utils import run_bass_kernel_spmd

F32 = mybir.dt.float32
F16 = mybir.dt.float16
AF = mybir.ActivationFunctionType

P = 128
NCORES = 8
C = 64          # total cases
CC = C // NCORES  # cases per core
CIN = 64        # caseNN input dim
H = 256         # caseNN hidden
HB = H // P     # h k-blocks (2)
DIN = 256       # domain feature dim (in = out = 256 for every layer)
IB = DIN // P   # 2
NL = 4          # layers
N = 2048        # samples
D = DIN * DIN   # 65536 flattened per-layer weight
DSH = D // NCORES  # 8192 per-core d shard
QD = DSH // 4   # 2048-wide quarters of the shard
GRP = 4         # cases per domain group
NGRP = CC // GRP
NSL = 2         # n-slots of 1024 per (case, layer, ob)
_nc_cache = {}


def _build():
    nc = bacc.Bacc("TRN2", target_bir_lowering=False, debug=False, num_devices=NCORES)

    # ---- per-core external I/O ----
    xt = nc.dram_tensor("xt", [P, IB, N], F16, kind="ExternalInput").ap()
    ot = nc.dram_tensor("ot", [P, C], F16, kind="ExternalInput").ap()
    oto = nc.dram_tensor("oto", [P, CC], F16, kind="ExternalInput").ap()
    wc = nc.dram_tensor("wc", [P, H], F16, kind="ExternalInput").ap()
    bc2 = nc.dram_tensor("bc2", [P, HB], F32, kind="ExternalInput").ap()
    wws = nc.dram_tensor("wws", [NL, H, DSH], F16, kind="ExternalInput").ap()
    wbT = nc.dram_tensor("wbT", [P, HB, NL, DIN], F16, kind="ExternalInput").ap()
    bbT = nc.dram_tensor("bbT", [P, IB, NL], F32, kind="ExternalInput").ap()
    bwT = nc.dram_tensor("bwT", [P, NL, IB, DIN], F16, kind="ExternalInput").ap()
    yt = nc.dram_tensor("yt", [CC, IB, P, N], F16, kind="ExternalOutput").ap()

    with tile.TileContext(nc) as tc:
        with (
            tc.tile_pool(name="const", bufs=1) as const,
            tc.tile_pool(name="dram", bufs=1, space="DRAM") as dram,
            tc.tile_pool(name="ww", bufs=6) as ww,
            tc.tile_pool(name="wstg", bufs=4) as wstg,
            tc.tile_pool(name="wt", bufs=16) as wtp,
            tc.tile_pool(name="act", bufs=1) as act,
            tc.tile_pool(name="outs", bufs=3) as outs,
        ):
            # caseNN + weight-gen psum (2 banks); closed before the domain
            # pool opens so the domain gets all 8 banks
            ps_w_ctx = tc.tile_pool(name="ps_w", bufs=2, space="PSUM")
            ps_w = ps_w_ctx.__enter__()
            # ---- dummy collective: absorb the one-time CC firmware cold
            # start (~40us) concurrently with weight-gen ----
            cc_warm_in = dram.tile([NCORES, 64], F16, name="cc_warm_in")
            cc_warm_out = dram.tile([NCORES, 64], F16, name="cc_warm_out")
            nc.gpsimd.collective_compute(
                "AllToAll",
                mybir.AluOpType.bypass,
                replica_groups=[list(range(NCORES))],
                ins=[cc_warm_in.opt()],
                outs=[cc_warm_out.opt()],
            )

            # ---- tiny consts first: keep the wgen(0) critical path clear ----
            wc_sb = const.tile([P, H], F16)
            nc.sync.dma_start(wc_sb[:], wc)
            bc_sb = const.tile([P, HB], F32)
            nc.sync.dma_start(bc_sb[:], bc2)
            ot_sb = const.tile([P, C], F16)
            nc.sync.dma_start(ot_sb[:], ot)
            oto_sb = const.tile([P, CC], F16)
            nc.sync.dma_start(oto_sb[:], oto)

            # ---- PE warm-up: drive HAM to K=8/8 before weight-gen ----
            warm = ps_w.tile([P, 512], F32, tag="psw", name="warm")
            for i in range(24):
                nc.tensor.matmul(warm[:, :256], lhsT=wc_sb[:, 0:P],
                                 rhs=wc_sb, start=True, stop=True)

            # ---- caseNN hidden: hT[h, c] = silu(Wc.T @ o.T + bc) ----
            hT_sb = const.tile([P, HB, C], F16)
            for kb in range(HB):
                ps = ps_w.tile([P, 512], F32, tag="psw", name="psh")[:, :C]
                nc.tensor.matmul(ps, lhsT=wc_sb[:, ts(kb, P)], rhs=ot_sb,
                                 start=True, stop=True)
                nc.scalar.activation(hT_sb[:, kb, :], ps, AF.Silu,
                                     bias=bc_sb[:, kb : kb + 1])

            # ---- DRAM staging for collectives: {l0+l1}, {l2}, {l3} so the
            # first AllToAll delivers two layers and the chain stays ahead
            # of the domain windows ----
            w_sh01 = dram.tile([C, 2 * DSH], F16, name="w_sh01")
            w_fl01 = dram.tile([C, 2 * DSH], F16, name="w_fl01")
            w_sh2 = dram.tile([C, DSH], F16, name="w_sh2")
            w_fl2 = dram.tile([C, DSH], F16, name="w_fl2")
            w_sh3 = dram.tile([C, DSH], F16, name="w_sh3")
            w_fl3 = dram.tile([C, DSH], F16, name="w_fl3")
            # rows: j*CC + c_loc (j = source core = d-shard index);
            # d global = i*256 + o, shard j covers i in [32j, 32j+32)
            v01 = w_fl01.rearrange("(j c) (l il o) -> l j c il o", c=CC, l=2, o=DIN)
            v2 = w_fl2.rearrange("(j c) (il o) -> j c il o", c=CC, o=DIN)
            v3 = w_fl3.rearrange("(j c) (il o) -> j c il o", c=CC, o=DIN)

            def wf_view(l, j, c):
                if l < 2:
                    return v01[l, j, c]
                return (v2 if l == 2 else v3)[j, c]

            def shard_dst(l, off, size):
                if l < 2:
                    return w_sh01[:, ds(l * DSH + off, size)]
                return (w_sh2 if l == 2 else w_sh3)[:, ds(off, size)]

            def wgen(l):
                """weight-gen layer l: w[c, d-shard] for all 64 cases,
                col-tiled M=64 matmul pairs into [128, 512] psum tiles."""
                wws_l = wws[l].rearrange("(kb p) d -> p kb d", p=P)
                for q in range(4):
                    wwt = ww.tile([P, HB, QD], F16, tag="wwt", name=f"wwt{l}{q}")
                    nc.sync.dma_start(wwt[:], wws_l[:, :, ts(q, QD)])
                    for pr in range(2):  # chunk-pairs (2 x 512) per quarter
                        ps = ps_w.tile([P, 512], F32, tag="psw", name=f"psw{l}{q}{pr}")
                        for kb in range(HB):
                            nc.tensor.matmul(
                                ps[0:64, :], lhsT=hT_sb[:, kb, :],
                                rhs=wwt[:, kb, ds(pr * 1024, 512)],
                                start=(kb == 0), stop=(kb == HB - 1),
                            )
                            nc.tensor.matmul(
                                ps[64:128, :], lhsT=hT_sb[:, kb, :],
                                rhs=wwt[:, kb, ds(pr * 1024 + 512, 512)],
                                start=(kb == 0), stop=(kb == HB - 1),
                            )
                        stg = wstg.tile([P, 512], F16, tag="wstg", name=f"stg{l}{q}{pr}")
                        nc.vector.tensor_copy(stg[:], ps)
                        base = q * QD + pr * 1024
                        nc.sync.dma_start(shard_dst(l, base, 512), stg[0:64, :])
                        nc.sync.dma_start(shard_dst(l, base + 512, 512), stg[64:128, :])
                if l != 0:
                    ins_t, outs_t = {1: (w_sh01, w_fl01), 2: (w_sh2, w_fl2),
                                     3: (w_sh3, w_fl3)}[l]
                    nc.gpsimd.collective_compute(
                        "AllToAll",
                        mybir.AluOpType.bypass,
                        replica_groups=[list(range(NCORES))],
                        ins=[ins_t.opt()],
                        outs=[outs_t.opt()],
                    )

            # domain state per case
            a_cur = [None] * CC   # input tile for next layer
            a_skip = [None] * CC  # a0 (skip accumulator, f16)
            wt_tiles = {}

            def prep(g, l):
                """DMA-gather + bias-add the domain weight tiles for group g
                layer l (4 cases x 2 ib-tiles of [128, 256] f16)."""
                # group 0 bias-adds on DVE (free early); group 1 on GpSimd
                # (collectives done by then) so l3 DVE drains never block them
                eng = nc.vector if g == 0 else nc.gpsimd
                for c in range(g * GRP, (g + 1) * GRP):
                    for ib in range(IB):
                        wt_t = wtp.tile([P, DIN], F16, tag="wt", name=f"wt{g}{l}{c}{ib}")
                        for jr in range(4):
                            j = 4 * ib + jr
                            nc.sync.dma_start(wt_t[ds(32 * jr, 32), :],
                                              wf_view(l, j, c))
                        eng.tensor_add(wt_t[:], wt_t[:], bwT_sb[:, l, ib, :])
                        wt_tiles[(c, ib)] = wt_t

            def compute(g, l):
                """domain layer l for the 4 cases of group g."""
                for c in range(g * GRP, (g + 1) * GRP):
                    a_prev = xt_sb if l == 0 else a_cur[c]
                    if l == 0:
                        a_new = act.tile([P, IB, N], F16, tag="a0", bufs=4,
                                         name=f"a0_{c}")
                    elif l < NL - 1:
                        a_new = act.tile([P, IB, N], F16, tag="ax", bufs=8,
                                         name=f"a_{c}_{l}")
                    else:
                        a_new = outs.tile([P, IB, N], F16, tag="out",
                                          name=f"o_{c}")
                    for ob in range(IB):
                        # one 4-bank psum tile per ob: each lhsT (stationary
                        # weight) covers 4 consecutive matmuls, and the whole
                        # 2048-wide tile drains in ONE ACT/DVE instruction
                        ps = ps_y.tile([P, 2048], F32, tag="psy",
                                       name=f"psy{c}{l}{ob}")
                        for ib in range(IB):
                            for nch in range(4):
                                nc.tensor.matmul(
                                    ps[:, ts(nch, 512)],
                                    lhsT=wt_tiles[(c, ib)][:, ts(ob, P)],
                                    rhs=a_prev[:, ib, ds(nch * 512, 512)],
                                    start=(ib == 0), stop=(ib == IB - 1),
                                )
                        dst = a_new[:, ob, :]
                        bias = bO_sb[:, ob, l, c : c + 1]
                        if l < NL - 1:
                            nc.scalar.activation(dst, ps, AF.Silu, bias=bias)
                        elif ob == 0:
                            nc.scalar.activation(dst, ps, AF.Identity, bias=bias)
                        else:
                            nc.vector.tensor_scalar_add(dst, ps, bias)
                        if l == NL - 1:
                            nc.sync.dma_start(yt[c, ob], a_new[:, ob, :])
                    if l == 0:
                        a_skip[c] = a_new
                    if l == 2:  # skip: a0 += a2, feeds final layer
                        nc.vector.tensor_add(a_skip[c][:], a_skip[c][:], a_new[:])
                        a_new = a_skip[c]
                    a_cur[c] = a_new

            # ---- braided issue: wgen layer l+1 interleaves domain layer l ----
            wgen(0)
            wgen(1)
            wgen(2)
            wgen(3)

            # big consts stream after all Ww quarters have queue priority
            xt_sb = const.tile([P, IB, N], F16)
            nc.sync.dma_start(xt_sb[:], xt)
            wbT_sb = const.tile([P, HB, NL, DIN], F16)
            nc.sync.dma_start(wbT_sb[:], wbT)
            bbT_sb = const.tile([P, IB, NL], F32)
            nc.sync.dma_start(bbT_sb[:], bbT)
            bwT_sb = const.tile([P, NL, IB, DIN], F16)
            nc.sync.dma_start(bwT_sb[:], bwT)

            # ---- caseNN for own cases: hTo + per-layer bias bO[o, ob, l, c] ----
            hTo_sb = const.tile([P, HB, CC], F16)
            for kb in range(HB):
                ps2 = ps_w.tile([P, 512], F32, tag="psw", name="psh2")[:, :CC]
                nc.tensor.matmul(ps2, lhsT=wc_sb[:, ts(kb, P)], rhs=oto_sb,
                                 start=True, stop=True)
                nc.scalar.activation(hTo_sb[:, kb, :], ps2, AF.Silu,
                                     bias=bc_sb[:, kb : kb + 1])
            bO_sb = const.tile([P, IB, NL, CC], F32)
            for l in range(NL):
                for ob in range(IB):
                    ps = ps_w.tile([P, 512], F32, tag="psw", name="psb")[:, :CC]
                    for kb in range(HB):
                        nc.tensor.matmul(
                            ps, lhsT=wbT_sb[:, kb, l, ts(ob, P)],
                            rhs=hTo_sb[:, kb, :],
                            start=(kb == 0), stop=(kb == HB - 1),
                        )
                    nc.scalar.activation(
                        bO_sb[:, ob, l, :], ps, AF.Identity,
                        bias=bbT_sb[:, ob, l : l + 1]
                    )

            # filler matmuls: keep the PE HAM-warm across the first
            # AllToAll wait so domain layer 0 starts at full clock
            fill = ps_w.tile([P, 512], F32, tag="psw", name="fill")
            for i in range(14):
                nc.tensor.matmul(fill[:, :256], lhsT=wc_sb[:, 0:P],
                                 rhs=wc_sb, start=True, stop=True)

            ps_w_ctx.__exit__(None, None, None)
            ps_y_ctx = tc.tile_pool(name="ps_y", bufs=2, space="PSUM")
            ps_y = ps_y_ctx.__enter__()
            for g in range(NGRP):
                for l in range(NL):
                    prep(g, l)
                    compute(g, l)
            ps_y_ctx.__exit__(None, None, None)

    nc.compile()
    return nc


def _prep_inputs(x, o, Wc, bc, Ww, bw, Wb, bb):
    x = np.asarray(x, np.float32)
    o = np.asarray(o, np.float32)
    Wc = np.asarray(Wc, np.float32)
    bc = np.asarray(bc, np.float32)
    Ww = np.asarray(Ww, np.float32)
    bw = np.asarray(bw, np.float32)
    Wb = np.asarray(Wb, np.float32)
    bb = np.asarray(bb, np.float32)

    xt = np.ascontiguousarray(x.T.reshape(IB, P, N).transpose(1, 0, 2)).astype(np.float16)
    otf = np.zeros((P, C), np.float16)
    otf[:CIN, :] = o.T
    wcp = np.zeros((P, H), np.float16)
    wcp[:CIN, :] = Wc
    bc2 = np.ascontiguousarray(bc.reshape(HB, P).T)
    wbT = np.ascontiguousarray(Wb.reshape(NL, HB, P, DIN).transpose(2, 1, 0, 3)).astype(np.float16)
    bbT = np.ascontiguousarray(bb.reshape(NL, IB, P).transpose(2, 1, 0))
    bwT = np.ascontiguousarray(bw.reshape(NL, IB, P, DIN).transpose(2, 0, 1, 3)).astype(np.float16)

    in_maps = []
    for k in range(NCORES):
        in_maps.append(
            {
                "xt": xt,
                "ot": otf,
                "oto": np.ascontiguousarray(otf[:, k * CC : (k + 1) * CC]),
                "wc": wcp,
                "bc2": bc2,
                "wws": np.ascontiguousarray(Ww[:, :, k * DSH : (k + 1) * DSH]).astype(np.float16),
                "wbT": wbT,
                "bbT": bbT,
                "bwT": bwT,
            }
        )
    return in_maps


def _run(inputs, trace=False):
    if "nc" not in _nc_cache:
        _nc_cache["nc"] = _build()
    nc = _nc_cache["nc"]
    in_maps = _prep_inputs(**inputs)
    res = run_bass_kernel_spmd(
        nc, in_maps, core_ids=list(range(NCORES)), trace=trace
    )
    # yt per core: [CC, IB, P, N] f16 -> [CC, N, IB*P] case-major
    parts = []
    for k in range(NCORES):
        ytk = res.results[k]["yt"].astype(np.float32)
        parts.append(ytk.transpose(0, 3, 1, 2).reshape(CC, N, DIN))
    out = np.concatenate(parts, axis=0).reshape(C * N, DIN)
    return out, res


def kernel(**inputs):
    out, _ = _run(inputs, trace=False)
    return out
